# revision 3
# baseline (speedup 1.0000x reference)
"""AttentionBlock kernel for 8 Trainium2 NeuronCores.

Sharding: one (batch, head) pair per core (B=2 x H=4 = 8 cores).
Each core computes, for its (b, h):
    qkT   = [wq|wk]^T @ x_b + [bq|bk]      packed [128, S]: rows 0:64 q, 64:128 k
    v     = x_b^T @ wv                     [S, 64]  (+ ones column -> [S, 65])
    S^T[j, i] = sum_d k[j,d] q[i,d]        (22 j-tiles of 128)
    E = exp(S^T * 0.125 - 3)               (ScalarE, from PSUM)
    resT[d, i] = sum_j v_aug[j, d] E[j, i] (PSUM accumulation, 65 rows;
                                            row 64 = softmax denominator l)
    outT[c, i] = sum_d w_out[d, c] resT[d, i]  [256, S] (unnormalized)
Host: out_b = sum_h (outT / l + (b_v @ w_out_h)[:, None]) + b_out[:, None] + x_b.

This kernel is ScalarE-bound: exp over S*SP elements costs ~(N+352)/1.2 ns
per ACTIVATE at 128 lanes. Structure choices serve Act occupancy:
- qkv projections are interleaved into i-block 0's groups so the first
  ACTIVATE fires as soon as projection chunk 0 lands (not after all 6).
- exp instructions are merged 2-groups-at-a-time via a PSUM ping-pong:
  one [128,2048] (4-bank) + one [128,1024] (2-bank) score tile alternate,
  amortizing the 352-cycle ACT pipeline fill (42 instrs instead of 66).
- The Exp table load (~2.7us) is hoisted to kernel start via a dummy act.
- q and k projections share one PSUM bank (col-group-packed matmuls) and
  one bias-add; score-pair matmuls run concurrently in disjoint PE row
  groups off duplicated q/k copies, targeting different PSUM banks.
Attention-path matmuls run fp16 (2-byte streaming, errors suppressed
through the diffuse softmax); out-projection runs float32r.
"""

import numpy as np

C = 256
S = 2744
SP = 2816  # 22 * 128
H = 4
DK = 64
NT = 22  # j tiles of 128
SVALID_LAST = S - 21 * 128  # 56 valid rows in last j-tile

# i blocks (query positions): only valid range [0, 2744)
IBLOCKS = [(0, 512), (512, 512), (1024, 512), (1536, 512), (2048, 512), (2560, 184)]
# s blocks for the qk projection: full padded range [0, 2816)
SBLOCKS = [(0, 512), (512, 512), (1024, 512), (1536, 512), (2048, 512), (2560, 256)]

# per-i-block slot pattern: each slot is a tuple of group indices sharing one
# ACTIVATE (2 groups -> big 4-bank psum tile, 1 group -> small 2-bank tile)
SLOTS = [(0, 1), (2,), (3, 4), (5,), (6, 7), (8,), (9, 10)]

_NC = None


def _build():
    from contextlib import ExitStack

    import concourse.bacc as bacc
    import concourse.tile as tile
    from concourse import mybir

    f32 = mybir.dt.float32
    fr = mybir.dt.float32r
    f16 = mybir.dt.float16
    Exp = mybir.ActivationFunctionType.Exp

    nc = bacc.Bacc("TRN2", target_bir_lowering=False)

    xT = nc.dram_tensor("xT", [C, S], f16, kind="ExternalInput")
    wq = nc.dram_tensor("wq", [C, DK], f16, kind="ExternalInput")
    wk = nc.dram_tensor("wk", [C, DK], f16, kind="ExternalInput")
    wv = nc.dram_tensor("wv", [C, DK], f16, kind="ExternalInput")
    bqk = nc.dram_tensor("bqk", [128, 1], f32, kind="ExternalInput")
    wo = nc.dram_tensor("wo", [DK, C], f32, kind="ExternalInput")

    out = nc.dram_tensor("out", [C, S], f32, kind="ExternalOutput")
    lsum = nc.dram_tensor("lsum", [1, S], f32, kind="ExternalOutput")

    with tile.TileContext(nc) as tc, ExitStack() as ctx:
        consts = ctx.enter_context(tc.tile_pool(name="consts", bufs=1))
        big = ctx.enter_context(tc.tile_pool(name="big", bufs=1))
        exbp = ctx.enter_context(tc.tile_pool(name="exb", bufs=3))
        exsp = ctx.enter_context(tc.tile_pool(name="exs", bufs=3))
        resp = ctx.enter_context(tc.tile_pool(name="resp", bufs=2))
        outp = ctx.enter_context(tc.tile_pool(name="outp", bufs=3))
        bigsc = ctx.enter_context(tc.tile_pool(name="bigsc", bufs=1, space="PSUM"))
        smallsc = ctx.enter_context(tc.tile_pool(name="smallsc", bufs=1, space="PSUM"))
        pvp = ctx.enter_context(tc.tile_pool(name="pvp", bufs=1, space="PSUM"))
        shp = ctx.enter_context(tc.tile_pool(name="shp", bufs=1, space="PSUM"))

        # ---- act-table preload: tiny exp on a const, first Scalar instr ----
        ebias_sb = consts.tile([128, 1], f32)
        nc.vector.memset(ebias_sb, -3.0)
        warm_act = consts.tile([1, 1], f16)
        nc.scalar.activation(out=warm_act, in_=ebias_sb[0:1, 0:1], func=Exp)

        # ---- weights / constants in SBUF (fp16 direct) ----
        w_sb = consts.tile([128, 2, 3 * DK], f16)
        for idx, w_dram in enumerate((wq, wk, wv)):
            nc.gpsimd.dma_start(
                out=w_sb[:, :, idx * DK : (idx + 1) * DK],
                in_=w_dram.rearrange("(c p) d -> p c d", p=128),
            )

        def wslice(idx, cc):
            return w_sb[:, cc, idx * DK : (idx + 1) * DK]

        wo_stage = consts.tile([DK, C], f32)
        nc.gpsimd.dma_start(out=wo_stage, in_=wo[:, :])
        wo_sb = consts.tile([DK, C], fr)
        nc.vector.tensor_copy(wo_sb, wo_stage)

        bqk_sb = consts.tile([128, 1], f32)
        nc.gpsimd.dma_start(out=bqk_sb, in_=bqk[:, :])

        # ---- x in SBUF (fp16 direct) ----
        x_sb = big.tile([128, 2, SP], f16)
        nc.vector.memset(x_sb[:, :, S:SP], 0.0)
        for off, w in SBLOCKS:
            for cc in range(2):
                wv_ = min(w, S - off) if off < S else 0
                if wv_ > 0:
                    eng = nc.sync if cc == 0 else nc.gpsimd
                    eng.dma_start(
                        out=x_sb[:, cc, off : off + wv_],
                        in_=xT[cc * 128 : (cc + 1) * 128, off : off + wv_],
                    )

        # ---- PE warm-up: ramp the tensor engine clock under the x DMA ----
        warm_in = consts.tile([128, 512], f16)
        nc.vector.memset(warm_in, 0.0)
        warm_ps = smallsc.tile([128, 1024], f32, tag="sc", name="warmps")
        for r in range(4):
            nc.tensor.matmul(
                warm_ps[:, :512],
                lhsT=warm_in[:, :128],
                rhs=warm_in,
                start=(r == 0),
                stop=(r == 3),
            )

        # ---- q/k/v projections (emitted interleaved into i-block 0) ----
        # qkT: rows 0:64 = q, rows 64:128 = k.  qkT2: rows 0:64 = k (dup),
        # rows 64:128 = q (dup) -- enables concurrent row-group score pairs.
        qkT = big.tile([128, SP], f16)
        qkT2 = big.tile([128, SP], f16)
        v_sb = big.tile([128, NT, DK + 1], f16)
        nc.vector.memset(v_sb[:, : NT - 1, DK : DK + 1], 1.0)
        nc.vector.memset(v_sb[:, NT - 1, DK : DK + 1], 0.0)
        nc.vector.memset(v_sb[:SVALID_LAST, NT - 1, DK : DK + 1], 1.0)

        def qk_chunk(sb):
            off, w = SBLOCKS[sb]
            ps = shp.tile([128, 512], f32, tag="sh", name="psqk")
            for cc in range(2):
                nc.tensor.matmul(
                    ps[0:64, :w],
                    lhsT=wslice(0, cc),
                    rhs=x_sb[:, cc, off : off + w],
                    start=(cc == 0),
                    stop=(cc == 1),
                    tile_position=(0, 0),
                )
                nc.tensor.matmul(
                    ps[64:128, :w],
                    lhsT=wslice(1, cc),
                    rhs=x_sb[:, cc, off : off + w],
                    start=(cc == 0),
                    stop=(cc == 1),
                    tile_position=(0, 64),
                )
            nc.vector.tensor_scalar_add(qkT[:, off : off + w], ps[:, :w], bqk_sb)
            # dups for row-group packing: qkT2 low = k, high = q
            nc.gpsimd.dma_start(
                out=qkT2[0:64, off : off + w], in_=qkT[64:128, off : off + w]
            )
            nc.gpsimd.dma_start(
                out=qkT2[64:128, off : off + w], in_=qkT[0:64, off : off + w]
            )

        def v_pair(p):
            psv = shp.tile([128, 512], f32, tag="sh", name="psv")
            for u in range(2):
                t = 2 * p + u
                for cc in range(2):
                    nc.tensor.matmul(
                        psv[:, u * DK : (u + 1) * DK],
                        lhsT=x_sb[:, cc, t * 128 : (t + 1) * 128],
                        rhs=wslice(2, cc),
                        start=(cc == 0),
                        stop=(cc == 1),
                    )
            nc.vector.tensor_copy(
                v_sb[:, 2 * p : 2 * p + 2, :DK],
                psv[:, : 2 * DK].rearrange("p (b w) -> p b w", b=2),
            )

        # slot index -> prologue work emitted right before that slot's scores
        # (only during the first processed i-block). qk chunk c covers j-tiles
        # 4c..4c+3 (groups 2c, 2c+1); v_pair(p) feeds AV(p).
        pre_slot = {
            0: [lambda: qk_chunk(1), lambda: v_pair(1)],
            1: [lambda: qk_chunk(2), lambda: v_pair(2), lambda: v_pair(3)],
            2: [lambda: qk_chunk(3), lambda: v_pair(4), lambda: v_pair(5)],
            3: [lambda: qk_chunk(4), lambda: v_pair(6)],
            4: [lambda: qk_chunk(5), lambda: v_pair(7), lambda: v_pair(8)],
            5: [lambda: v_pair(9)],
            6: [lambda: v_pair(10)],
        }

        qk_chunk(0)
        v_pair(0)

        # ---- main attention loop ----
        def emit_scores(ps, colb, g, ioff, iw):
            for u in range(2):
                t = 2 * g + u
                lo, hi = (0, 64) if u == 0 else (64, 128)
                kt = qkT2 if u == 0 else qkT
                qt = qkT if u == 0 else qkT2
                nc.tensor.matmul(
                    ps[:, colb + u * 512 : colb + u * 512 + iw],
                    lhsT=kt[lo:hi, t * 128 : (t + 1) * 128],
                    rhs=qt[lo:hi, ioff : ioff + iw],
                    start=True,
                    stop=True,
                    tile_position=(lo, 0),
                )

        def emit_av(pv, ex, base, g, iw, is_first, is_last):
            for u in range(2):
                t = 2 * g + u
                nc.tensor.matmul(
                    pv[:, :iw],
                    lhsT=v_sb[:, t, :],
                    rhs=ex[:, base + u * 512 : base + u * 512 + iw],
                    start=(is_first and u == 0),
                    stop=(is_last and u == 1),
                )

        pending_tail = None
        for ib, (ioff, iw) in enumerate(IBLOCKS):
            pv = pvp.tile([DK + 1, 512], f32, tag="pv", name="pv")
            pending_av = []  # (ex_tile, base, g)
            for sidx, groups in enumerate(SLOTS):
                if ib == 0:
                    for work in pre_slot.get(sidx, ()):
                        work()
                nb = len(groups)
                if nb == 2:
                    ps = bigsc.tile([128, 2048], f32, tag="bsc", name="bsc")
                    ex = exbp.tile([128, 2048], f16, tag="exb", name="exb")
                else:
                    ps = smallsc.tile([128, 1024], f32, tag="sc", name="ssc")
                    ex = exsp.tile([128, 1024], f16, tag="exs", name="exs")
                for k, g in enumerate(groups):
                    emit_scores(ps, k * 1024, g, ioff, iw)
                ps3 = ps.rearrange("p (b w) -> p b w", b=2 * nb)[:, :, :iw]
                ex3 = ex.rearrange("p (b w) -> p b w", b=2 * nb)[:, :, :iw]
                nc.scalar.activation(
                    out=ex3, in_=ps3, func=Exp, bias=ebias_sb, scale=0.125
                )
                if sidx == 1 and pending_tail is not None:
                    pending_tail()
                    pending_tail = None
                for k, g in enumerate(groups):
                    pending_av.append((ex, k * 1024, g))
                while len(pending_av) > nb:
                    pex, base, pg = pending_av.pop(0)
                    emit_av(pv, pex, base, pg, iw, pg == 0, pg == 10)
            for pex, base, pg in pending_av:
                emit_av(pv, pex, base, pg, iw, pg == 0, pg == 10)

            res_sb = resp.tile([DK + 1, 512], fr, tag="res", name="res_sb")
            nc.vector.tensor_copy(res_sb[:, :iw], pv[:, :iw])
            nc.gpsimd.dma_start(
                out=lsum[0:1, ioff : ioff + iw],
                in_=res_sb[DK : DK + 1, :iw].bitcast(f32),
            )

            def tail(ioff=ioff, iw=iw, res_sb=res_sb):
                for cc in range(2):
                    po = shp.tile([128, 512], f32, tag="sh", name="po")
                    nc.tensor.matmul(
                        po[:, :iw],
                        lhsT=wo_sb[:, cc * 128 : (cc + 1) * 128],
                        rhs=res_sb[:DK, :iw],
                        start=True,
                        stop=True,
                    )
                    ob = outp.tile([128, 512], f32, tag="ob", name="ob")
                    nc.vector.tensor_copy(ob[:, :iw], po[:, :iw])
                    nc.sync.dma_start(
                        out=out[cc * 128 : (cc + 1) * 128, ioff : ioff + iw],
                        in_=ob[:, :iw],
                    )

            if ib == len(IBLOCKS) - 1:
                tail()
            else:
                pending_tail = tail

    nc.compile()
    return nc


def _get_nc():
    global _NC
    if _NC is None:
        _NC = _build()
    return _NC


def _make_in_maps(inputs):
    x = np.asarray(inputs["x"], dtype=np.float32)
    w_proj = np.asarray(inputs["w_proj"], dtype=np.float32)
    b_proj = np.asarray(inputs["b_proj"], dtype=np.float32)
    w_out = np.asarray(inputs["w_out"], dtype=np.float32)
    in_maps = []
    for core in range(8):
        b, h = divmod(core, H)
        base = h * 3 * DK
        bq = b_proj[base : base + DK]
        bk = b_proj[base + DK : base + 2 * DK]
        in_maps.append(
            {
                "xT": np.ascontiguousarray(x[b].reshape(C, S).astype(np.float16)),
                "wq": np.ascontiguousarray(
                    w_proj[:, base : base + DK].astype(np.float16)
                ),
                "wk": np.ascontiguousarray(
                    w_proj[:, base + DK : base + 2 * DK].astype(np.float16)
                ),
                "wv": np.ascontiguousarray(
                    w_proj[:, base + 2 * DK : base + 3 * DK].astype(np.float16)
                ),
                "bqk": np.ascontiguousarray(
                    np.concatenate([bq, bk]).reshape(128, 1).astype(np.float32)
                ),
                "wo": np.ascontiguousarray(w_out[h * DK : (h + 1) * DK, :]),
            }
        )
    return in_maps


def kernel(x, w_proj, b_proj, w_out, b_out):
    from concourse.bass_utils import run_bass_kernel_spmd

    x = np.asarray(x, dtype=np.float32)
    w_proj = np.asarray(w_proj, dtype=np.float32)
    b_proj = np.asarray(b_proj, dtype=np.float32)
    w_out = np.asarray(w_out, dtype=np.float32)
    b_out = np.asarray(b_out, dtype=np.float32)

    B = x.shape[0]
    nc = _get_nc()

    in_maps = _make_in_maps(
        {"x": x, "w_proj": w_proj, "b_proj": b_proj, "w_out": w_out, "b_out": b_out}
    )
    res = run_bass_kernel_spmd(nc, in_maps, list(range(8)))

    outs = np.zeros((B, C, S), dtype=np.float32)
    for b in range(B):
        acc = x[b].reshape(C, S).astype(np.float32) + b_out[:, None]
        for h in range(H):
            core = b * H + h
            dev_o = res.results[core]["out"]  # [C, S] unnormalized
            l = res.results[core]["lsum"]  # [1, S]
            bv = b_proj[h * 3 * DK + 2 * DK : h * 3 * DK + 3 * DK]
            corr = bv @ w_out[h * DK : (h + 1) * DK, :]  # [C]
            acc = acc + dev_o / l + corr[:, None]
        outs[b] = acc
    return outs.reshape(B, C, 14, 14, 14)


# revision 4
# speedup vs baseline: 1.4029x; 1.4029x over previous
"""AttentionBlock kernel for 8 Trainium2 NeuronCores.

Sharding: one (batch, head) pair per core (B=2 x H=4 = 8 cores).
Each core computes, for its (b, h):
    qkT   = [wq|wk]^T @ x_b + [bq|bk]      packed [128, S]: rows 0:64 q, 64:128 k
    v     = x_b^T @ wv                     [S, 64]  (+ ones column -> [S, 65])
    S^T[j, i] = sum_d k[j,d] q[i,d]        (22 j-tiles of 128)
    E = exp(S^T * 0.125 - 3)               (ScalarE, from PSUM)
    resT[d, i] = sum_j v_aug[j, d] E[j, i] (PSUM accumulation, 65 rows;
                                            row 64 = softmax denominator l)
    outT[c, i] = sum_d w_out[d, c] resT[d, i]  [256, S] (unnormalized)
Host: out_b = sum_h (outT / l + (b_v @ w_out_h)[:, None]) + b_out[:, None] + x_b.

This kernel is ScalarE-bound: exp over S*SP elements costs ~(N+352)/1.2 ns
per ACTIVATE at 128 lanes. Structure choices serve Act occupancy:
- qkv projections are interleaved into i-block 0's groups so the first
  ACTIVATE fires as soon as projection chunk 0 lands (not after all 6).
- exp instructions are merged 2-groups-at-a-time via a PSUM ping-pong:
  one [128,2048] (4-bank) + one [128,1024] (2-bank) score tile alternate,
  amortizing the 352-cycle ACT pipeline fill (42 instrs instead of 66).
- The Exp table load (~2.7us) is hoisted to kernel start via a dummy act.
- q and k projections share one PSUM bank (col-group-packed matmuls) and
  one bias-add; score-pair matmuls run concurrently in disjoint PE row
  groups off duplicated q/k copies, targeting different PSUM banks.
Attention-path matmuls run fp16 (2-byte streaming, errors suppressed
through the diffuse softmax); out-projection runs float32r.
"""

import numpy as np

C = 256
S = 2744
SP = 2816  # 22 * 128
H = 4
DK = 64
NT = 22  # j tiles of 128
SVALID_LAST = S - 21 * 128  # 56 valid rows in last j-tile

# i blocks (query positions): only valid range [0, 2744)
IBLOCKS = [(0, 512), (512, 512), (1024, 512), (1536, 512), (2048, 512), (2560, 184)]
# s blocks for the qk projection: full padded range [0, 2816)
SBLOCKS = [(0, 512), (512, 512), (1024, 512), (1536, 512), (2048, 512), (2560, 256)]

NG = 11  # j-tile-pair groups per i-block

_NC = None


def _build():
    from contextlib import ExitStack

    import concourse.bacc as bacc
    import concourse.tile as tile
    from concourse import mybir

    f32 = mybir.dt.float32
    fr = mybir.dt.float32r
    f16 = mybir.dt.float16
    Exp = mybir.ActivationFunctionType.Exp

    nc = bacc.Bacc("TRN2", target_bir_lowering=False)

    xT = nc.dram_tensor("xT", [C, S], f16, kind="ExternalInput")
    wq = nc.dram_tensor("wq", [C, DK], f16, kind="ExternalInput")
    wk = nc.dram_tensor("wk", [C, DK], f16, kind="ExternalInput")
    wv = nc.dram_tensor("wv", [C, DK], f16, kind="ExternalInput")
    bqk = nc.dram_tensor("bqk", [128, 1], f32, kind="ExternalInput")
    wo = nc.dram_tensor("wo", [DK, C], f32, kind="ExternalInput")

    out = nc.dram_tensor("out", [C, S], f32, kind="ExternalOutput")
    lsum = nc.dram_tensor("lsum", [1, S], f32, kind="ExternalOutput")

    with tile.TileContext(nc) as tc, ExitStack() as ctx:
        consts = ctx.enter_context(tc.tile_pool(name="consts", bufs=1))
        big = ctx.enter_context(tc.tile_pool(name="big", bufs=1))
        expp = ctx.enter_context(tc.tile_pool(name="expp", bufs=6))
        resp = ctx.enter_context(tc.tile_pool(name="resp", bufs=2))
        outp = ctx.enter_context(tc.tile_pool(name="outp", bufs=3))
        scp = ctx.enter_context(tc.tile_pool(name="scp", bufs=3, space="PSUM"))
        pvp = ctx.enter_context(tc.tile_pool(name="pvp", bufs=1, space="PSUM"))
        shp = ctx.enter_context(tc.tile_pool(name="shp", bufs=1, space="PSUM"))

        # ---- act-table preload: tiny exp on a const, first Scalar instr ----
        ebias_sb = consts.tile([128, 1], f32)
        nc.vector.memset(ebias_sb, -3.0)
        warm_act = consts.tile([1, 1], f16)
        nc.scalar.activation(out=warm_act, in_=ebias_sb[0:1, 0:1], func=Exp)

        # ---- weights / constants in SBUF (fp16 direct) ----
        w_sb = consts.tile([128, 2, 3 * DK], f16)
        for idx, w_dram in enumerate((wq, wk, wv)):
            nc.gpsimd.dma_start(
                out=w_sb[:, :, idx * DK : (idx + 1) * DK],
                in_=w_dram.rearrange("(c p) d -> p c d", p=128),
            )

        def wslice(idx, cc):
            return w_sb[:, cc, idx * DK : (idx + 1) * DK]

        wo_stage = consts.tile([DK, C], f32)
        nc.gpsimd.dma_start(out=wo_stage, in_=wo[:, :])
        wo_sb = consts.tile([DK, C], fr)
        nc.vector.tensor_copy(wo_sb, wo_stage)

        bqk_sb = consts.tile([128, 1], f32)
        nc.gpsimd.dma_start(out=bqk_sb, in_=bqk[:, :])

        # ---- x in SBUF (fp16 direct) ----
        x_sb = big.tile([128, 2, SP], f16)
        nc.vector.memset(x_sb[:, :, S:SP], 0.0)
        for off, w in SBLOCKS:
            for cc in range(2):
                wv_ = min(w, S - off) if off < S else 0
                if wv_ > 0:
                    eng = nc.sync if cc == 0 else nc.gpsimd
                    eng.dma_start(
                        out=x_sb[:, cc, off : off + wv_],
                        in_=xT[cc * 128 : (cc + 1) * 128, off : off + wv_],
                    )

        # ---- PE warm-up: ramp the tensor engine clock under the x DMA ----
        warm_in = consts.tile([128, 512], f16)
        nc.vector.memset(warm_in, 0.0)
        warm_ps = scp.tile([128, 1024], f32, tag="sc", name="warmps")
        for r in range(4):
            nc.tensor.matmul(
                warm_ps[:, :512],
                lhsT=warm_in[:, :128],
                rhs=warm_in,
                start=(r == 0),
                stop=(r == 3),
            )

        # ---- q/k/v projections (emitted interleaved into i-block 0) ----
        # qkT: rows 0:64 = q, rows 64:128 = k.  qkT2: rows 0:64 = k (dup),
        # rows 64:128 = q (dup) -- enables concurrent row-group score pairs.
        qkT = big.tile([128, SP], f16)
        qkT2 = big.tile([128, SP], f16)
        v_sb = big.tile([128, NT, DK + 1], f16)
        nc.vector.memset(v_sb[:, : NT - 1, DK : DK + 1], 1.0)
        nc.vector.memset(v_sb[:, NT - 1, DK : DK + 1], 0.0)
        nc.vector.memset(v_sb[:SVALID_LAST, NT - 1, DK : DK + 1], 1.0)

        def qk_chunk(sb):
            off, w = SBLOCKS[sb]
            ps = shp.tile([128, 512], f32, tag="sh", name="psqk")
            for cc in range(2):
                nc.tensor.matmul(
                    ps[0:64, :w],
                    lhsT=wslice(0, cc),
                    rhs=x_sb[:, cc, off : off + w],
                    start=(cc == 0),
                    stop=(cc == 1),
                    tile_position=(0, 0),
                )
                nc.tensor.matmul(
                    ps[64:128, :w],
                    lhsT=wslice(1, cc),
                    rhs=x_sb[:, cc, off : off + w],
                    start=(cc == 0),
                    stop=(cc == 1),
                    tile_position=(0, 64),
                )
            nc.vector.tensor_scalar_add(qkT[:, off : off + w], ps[:, :w], bqk_sb)
            # dups for row-group packing: qkT2 low = k, high = q
            nc.gpsimd.dma_start(
                out=qkT2[0:64, off : off + w], in_=qkT[64:128, off : off + w]
            )
            nc.gpsimd.dma_start(
                out=qkT2[64:128, off : off + w], in_=qkT[0:64, off : off + w]
            )

        def v_pair(p):
            psv = shp.tile([128, 512], f32, tag="sh", name="psv")
            for u in range(2):
                t = 2 * p + u
                for cc in range(2):
                    nc.tensor.matmul(
                        psv[:, u * DK : (u + 1) * DK],
                        lhsT=x_sb[:, cc, t * 128 : (t + 1) * 128],
                        rhs=wslice(2, cc),
                        start=(cc == 0),
                        stop=(cc == 1),
                    )
            nc.vector.tensor_copy(
                v_sb[:, 2 * p : 2 * p + 2, :DK],
                psv[:, : 2 * DK].rearrange("p (b w) -> p b w", b=2),
            )

        # group g -> prefetch work emitted right after group g's activation
        # (first processed i-block only). qk chunk c covers j-tiles 4c..4c+3
        # (scores of groups 2c, 2c+1); v_pair(p) feeds AV(p) at group p+1.
        pre_slot = {
            0: [lambda: qk_chunk(1), lambda: v_pair(1)],
            1: [lambda: qk_chunk(2), lambda: v_pair(2)],
            2: [lambda: qk_chunk(3), lambda: v_pair(3), lambda: v_pair(4)],
            3: [lambda: qk_chunk(4), lambda: v_pair(5), lambda: v_pair(6)],
            4: [lambda: qk_chunk(5), lambda: v_pair(7)],
            5: [lambda: v_pair(8)],
            6: [lambda: v_pair(9)],
            7: [lambda: v_pair(10)],
        }

        qk_chunk(0)
        v_pair(0)

        # ---- main attention loop ----
        def emit_scores(ps, colb, g, ioff, iw):
            for u in range(2):
                t = 2 * g + u
                lo, hi = (0, 64) if u == 0 else (64, 128)
                kt = qkT2 if u == 0 else qkT
                qt = qkT if u == 0 else qkT2
                nc.tensor.matmul(
                    ps[:, colb + u * 512 : colb + u * 512 + iw],
                    lhsT=kt[lo:hi, t * 128 : (t + 1) * 128],
                    rhs=qt[lo:hi, ioff : ioff + iw],
                    start=True,
                    stop=True,
                    tile_position=(lo, 0),
                )

        def emit_av(pv, ex, base, g, iw, is_first, is_last):
            for u in range(2):
                t = 2 * g + u
                nc.tensor.matmul(
                    pv[:, :iw],
                    lhsT=v_sb[:, t, :],
                    rhs=ex[:, base + u * 512 : base + u * 512 + iw],
                    start=(is_first and u == 0),
                    stop=(is_last and u == 1),
                )

        pending_tail = None
        for ib, (ioff, iw) in enumerate(IBLOCKS):
            pv = pvp.tile([DK + 1, 512], f32, tag="pv", name="pv")
            pending_av = []  # (ex_tile, g)
            for g in range(NG):
                ps = scp.tile([128, 1024], f32, tag="sc", name="sc")
                ex = expp.tile([128, 1024], f16, tag="ex", name="ex")
                emit_scores(ps, 0, g, ioff, iw)
                ps3 = ps.rearrange("p (b w) -> p b w", b=2)[:, :, :iw]
                ex3 = ex.rearrange("p (b w) -> p b w", b=2)[:, :, :iw]
                nc.scalar.activation(
                    out=ex3, in_=ps3, func=Exp, bias=ebias_sb, scale=0.125
                )
                if ib == 0:
                    for work in pre_slot.get(g, ()):
                        work()
                if g == 1 and pending_tail is not None:
                    pending_tail()
                    pending_tail = None
                pending_av.append((ex, g))
                if len(pending_av) > 1:
                    pex, pg = pending_av.pop(0)
                    emit_av(pv, pex, 0, pg, iw, pg == 0, pg == NG - 1)
            for pex, pg in pending_av:
                emit_av(pv, pex, 0, pg, iw, pg == 0, pg == NG - 1)

            res_sb = resp.tile([DK + 1, 512], fr, tag="res", name="res_sb")
            nc.vector.tensor_copy(res_sb[:, :iw], pv[:, :iw])
            nc.gpsimd.dma_start(
                out=lsum[0:1, ioff : ioff + iw],
                in_=res_sb[DK : DK + 1, :iw].bitcast(f32),
            )

            def tail(ioff=ioff, iw=iw, res_sb=res_sb):
                for cc in range(2):
                    po = shp.tile([128, 512], f32, tag="sh", name="po")
                    nc.tensor.matmul(
                        po[:, :iw],
                        lhsT=wo_sb[:, cc * 128 : (cc + 1) * 128],
                        rhs=res_sb[:DK, :iw],
                        start=True,
                        stop=True,
                    )
                    ob = outp.tile([128, 512], f32, tag="ob", name="ob")
                    nc.vector.tensor_copy(ob[:, :iw], po[:, :iw])
                    nc.sync.dma_start(
                        out=out[cc * 128 : (cc + 1) * 128, ioff : ioff + iw],
                        in_=ob[:, :iw],
                    )

            if ib == len(IBLOCKS) - 1:
                tail()
            else:
                pending_tail = tail

    nc.compile()
    return nc


def _get_nc():
    global _NC
    if _NC is None:
        _NC = _build()
    return _NC


def _make_in_maps(inputs):
    x = np.asarray(inputs["x"], dtype=np.float32)
    w_proj = np.asarray(inputs["w_proj"], dtype=np.float32)
    b_proj = np.asarray(inputs["b_proj"], dtype=np.float32)
    w_out = np.asarray(inputs["w_out"], dtype=np.float32)
    in_maps = []
    for core in range(8):
        b, h = divmod(core, H)
        base = h * 3 * DK
        bq = b_proj[base : base + DK]
        bk = b_proj[base + DK : base + 2 * DK]
        in_maps.append(
            {
                "xT": np.ascontiguousarray(x[b].reshape(C, S).astype(np.float16)),
                "wq": np.ascontiguousarray(
                    w_proj[:, base : base + DK].astype(np.float16)
                ),
                "wk": np.ascontiguousarray(
                    w_proj[:, base + DK : base + 2 * DK].astype(np.float16)
                ),
                "wv": np.ascontiguousarray(
                    w_proj[:, base + 2 * DK : base + 3 * DK].astype(np.float16)
                ),
                "bqk": np.ascontiguousarray(
                    np.concatenate([bq, bk]).reshape(128, 1).astype(np.float32)
                ),
                "wo": np.ascontiguousarray(w_out[h * DK : (h + 1) * DK, :]),
            }
        )
    return in_maps


def kernel(x, w_proj, b_proj, w_out, b_out):
    from concourse.bass_utils import run_bass_kernel_spmd

    x = np.asarray(x, dtype=np.float32)
    w_proj = np.asarray(w_proj, dtype=np.float32)
    b_proj = np.asarray(b_proj, dtype=np.float32)
    w_out = np.asarray(w_out, dtype=np.float32)
    b_out = np.asarray(b_out, dtype=np.float32)

    B = x.shape[0]
    nc = _get_nc()

    in_maps = _make_in_maps(
        {"x": x, "w_proj": w_proj, "b_proj": b_proj, "w_out": w_out, "b_out": b_out}
    )
    res = run_bass_kernel_spmd(nc, in_maps, list(range(8)))

    outs = np.zeros((B, C, S), dtype=np.float32)
    for b in range(B):
        acc = x[b].reshape(C, S).astype(np.float32) + b_out[:, None]
        for h in range(H):
            core = b * H + h
            dev_o = res.results[core]["out"]  # [C, S] unnormalized
            l = res.results[core]["lsum"]  # [1, S]
            bv = b_proj[h * 3 * DK + 2 * DK : h * 3 * DK + 3 * DK]
            corr = bv @ w_out[h * DK : (h + 1) * DK, :]  # [C]
            acc = acc + dev_o / l + corr[:, None]
        outs[b] = acc
    return outs.reshape(B, C, 14, 14, 14)


# revision 6
# speedup vs baseline: 1.4543x; 1.0366x over previous
"""AttentionBlock kernel for 8 Trainium2 NeuronCores.

Sharding: one (batch, head) pair per core (B=2 x H=4 = 8 cores).
Each core computes, for its (b, h):
    qkT   = [wq|wk]^T @ x_b + [bq|bk]      packed [128, S]: rows 0:64 q, 64:128 k
    v     = x_b^T @ wv                     [S, 64]  (+ ones column -> [S, 65])
    S^T[j, i] = sum_d k[j,d] q[i,d]        (22 j-tiles of 128)
    E = exp(S^T * 0.125 - 3)               (ScalarE, from PSUM)
    resT[d, i] = sum_j v_aug[j, d] E[j, i] (PSUM accumulation, 65 rows;
                                            row 64 = softmax denominator l)
    outT[c, i] = sum_d w_out[d, c] resT[d, i]  [256, S] (unnormalized)
Host: out_b = sum_h (outT / l + (b_v @ w_out_h)[:, None]) + b_out[:, None] + x_b.

This kernel is ScalarE-bound: exp over S*SP elements costs ~(N+352)/1.2 ns
per ACTIVATE at 128 lanes. Structure choices serve Act occupancy:
- qkv projections are interleaved into i-block 0's groups so the first
  ACTIVATE fires as soon as projection chunk 0 lands (not after all 6).
- exp instructions are merged 2-groups-at-a-time via a PSUM ping-pong:
  one [128,2048] (4-bank) + one [128,1024] (2-bank) score tile alternate,
  amortizing the 352-cycle ACT pipeline fill (42 instrs instead of 66).
- The Exp table load (~2.7us) is hoisted to kernel start via a dummy act.
- q and k projections share one PSUM bank (col-group-packed matmuls) and
  one bias-add; score-pair matmuls run concurrently in disjoint PE row
  groups off duplicated q/k copies, targeting different PSUM banks.
Attention-path matmuls run fp16 (2-byte streaming, errors suppressed
through the diffuse softmax); out-projection runs float32r.
"""

import numpy as np

C = 256
S = 2744
SP = 2816  # 22 * 128
H = 4
DK = 64
NT = 22  # j tiles of 128
SVALID_LAST = S - 21 * 128  # 56 valid rows in last j-tile

# i blocks (query positions): only valid range [0, 2744)
IBLOCKS = [(0, 512), (512, 512), (1024, 512), (1536, 512), (2048, 512), (2560, 184)]
# s blocks for the qk projection: full padded range [0, 2816)
SBLOCKS = [(0, 512), (512, 512), (1024, 512), (1536, 512), (2048, 512), (2560, 256)]

NG = 11  # j-tile-pair groups per i-block

_NC = None


def _build():
    from contextlib import ExitStack

    import concourse.bacc as bacc
    import concourse.tile as tile
    from concourse import mybir

    f32 = mybir.dt.float32
    fr = mybir.dt.float32r
    f16 = mybir.dt.float16
    Exp = mybir.ActivationFunctionType.Exp

    nc = bacc.Bacc("TRN2", target_bir_lowering=False)

    xT = nc.dram_tensor("xT", [C, S], f16, kind="ExternalInput")
    wq = nc.dram_tensor("wq", [C, DK], f16, kind="ExternalInput")
    wk = nc.dram_tensor("wk", [C, DK], f16, kind="ExternalInput")
    wv = nc.dram_tensor("wv", [C, DK], f16, kind="ExternalInput")
    bqk = nc.dram_tensor("bqk", [128, 1], f32, kind="ExternalInput")
    bkq = nc.dram_tensor("bkq", [128, 1], f32, kind="ExternalInput")
    wo = nc.dram_tensor("wo", [DK, C], f32, kind="ExternalInput")

    out = nc.dram_tensor("out", [C, S], f32, kind="ExternalOutput")
    lsum = nc.dram_tensor("lsum", [1, S], f32, kind="ExternalOutput")

    with tile.TileContext(nc) as tc, ExitStack() as ctx:
        consts = ctx.enter_context(tc.tile_pool(name="consts", bufs=1))
        big = ctx.enter_context(tc.tile_pool(name="big", bufs=1))
        expp = ctx.enter_context(tc.tile_pool(name="expp", bufs=6))
        resp = ctx.enter_context(tc.tile_pool(name="resp", bufs=2))
        outp = ctx.enter_context(tc.tile_pool(name="outp", bufs=3))
        scp = ctx.enter_context(tc.tile_pool(name="scp", bufs=3, space="PSUM"))
        pvp = ctx.enter_context(tc.tile_pool(name="pvp", bufs=1, space="PSUM"))
        shp = ctx.enter_context(tc.tile_pool(name="shp", bufs=1, space="PSUM"))

        # ---- act-table preload: tiny exp on a const, first Scalar instr ----
        ebias_sb = consts.tile([128, 1], f32)
        nc.vector.memset(ebias_sb, -3.0)
        warm_act = consts.tile([1, 1], f16)
        nc.scalar.activation(out=warm_act, in_=ebias_sb[0:1, 0:1], func=Exp)

        # ---- weights / constants in SBUF (fp16 direct) ----
        w_sb = consts.tile([128, 2, 3 * DK], f16)
        for idx, w_dram in enumerate((wq, wk, wv)):
            nc.gpsimd.dma_start(
                out=w_sb[:, :, idx * DK : (idx + 1) * DK],
                in_=w_dram.rearrange("(c p) d -> p c d", p=128),
            )

        def wslice(idx, cc):
            return w_sb[:, cc, idx * DK : (idx + 1) * DK]

        wo_stage = consts.tile([DK, C], f32)
        nc.gpsimd.dma_start(out=wo_stage, in_=wo[:, :])
        wo_sb = consts.tile([DK, C], fr)
        nc.vector.tensor_copy(wo_sb, wo_stage)

        bqk_sb = consts.tile([128, 1], f32)
        nc.gpsimd.dma_start(out=bqk_sb, in_=bqk[:, :])
        bkq_sb = consts.tile([128, 1], f32)
        nc.gpsimd.dma_start(out=bkq_sb, in_=bkq[:, :])

        # ---- x in SBUF (fp16 direct) ----
        x_sb = big.tile([128, 2, SP], f16)
        nc.vector.memset(x_sb[:, :, S:SP], 0.0)
        for off, w in SBLOCKS:
            for cc in range(2):
                wv_ = min(w, S - off) if off < S else 0
                if wv_ > 0:
                    eng = nc.sync
                    eng.dma_start(
                        out=x_sb[:, cc, off : off + wv_],
                        in_=xT[cc * 128 : (cc + 1) * 128, off : off + wv_],
                    )

        # ---- PE warm-up: ramp the tensor engine clock under the x DMA ----
        warm_in = consts.tile([128, 512], f16)
        nc.vector.memset(warm_in, 0.0)
        warm_ps = scp.tile([128, 1024], f32, tag="sc", name="warmps")
        for r in range(4):
            nc.tensor.matmul(
                warm_ps[:, :512],
                lhsT=warm_in[:, :128],
                rhs=warm_in,
                start=(r == 0),
                stop=(r == 3),
            )

        # ---- q/k/v projections (emitted interleaved into i-block 0) ----
        # qkT: rows 0:64 = q, rows 64:128 = k.  qkT2: rows 0:64 = k (dup),
        # rows 64:128 = q (dup) -- enables concurrent row-group score pairs.
        qkT = big.tile([128, SP], f16)
        qkT2 = big.tile([128, SP], f16)
        v_sb = big.tile([128, NT, DK + 1], f16)
        nc.vector.memset(v_sb[:, : NT - 1, DK : DK + 1], 1.0)
        nc.vector.memset(v_sb[:, NT - 1, DK : DK + 1], 0.0)
        nc.vector.memset(v_sb[:SVALID_LAST, NT - 1, DK : DK + 1], 1.0)

        def qk_proj(sb, dst, bias_sb, lo_widx, hi_widx):
            # project into dst[:, chunk]: rows 0:64 <- lo_widx head, rows
            # 64:128 <- hi_widx head (col-group-packed concurrent matmuls)
            off, w = SBLOCKS[sb]
            ps = shp.tile([128, 512], f32, tag="sh", name="psqk")
            for cc in range(2):
                nc.tensor.matmul(
                    ps[0:64, :w],
                    lhsT=wslice(lo_widx, cc),
                    rhs=x_sb[:, cc, off : off + w],
                    start=(cc == 0),
                    stop=(cc == 1),
                    tile_position=(0, 0),
                )
                nc.tensor.matmul(
                    ps[64:128, :w],
                    lhsT=wslice(hi_widx, cc),
                    rhs=x_sb[:, cc, off : off + w],
                    start=(cc == 0),
                    stop=(cc == 1),
                    tile_position=(0, 64),
                )
            nc.vector.tensor_scalar_add(dst[:, off : off + w], ps[:, :w], bias_sb)

        def qk_chunk(sb):
            off, w = SBLOCKS[sb]
            qk_proj(sb, qkT, bqk_sb, 0, 1)
            if sb == 0:
                # critical path: build the swapped layout directly on the PE
                # instead of waiting for two SBUF->SBUF dup DMA round-trips
                qk_proj(sb, qkT2, bkq_sb, 1, 0)
            else:
                # dups for row-group packing: qkT2 low = k, high = q
                nc.gpsimd.dma_start(
                    out=qkT2[0:64, off : off + w], in_=qkT[64:128, off : off + w]
                )
                nc.gpsimd.dma_start(
                    out=qkT2[64:128, off : off + w], in_=qkT[0:64, off : off + w]
                )

        def v_pair(p):
            psv = shp.tile([128, 512], f32, tag="sh", name="psv")
            for u in range(2):
                t = 2 * p + u
                for cc in range(2):
                    nc.tensor.matmul(
                        psv[:, u * DK : (u + 1) * DK],
                        lhsT=x_sb[:, cc, t * 128 : (t + 1) * 128],
                        rhs=wslice(2, cc),
                        start=(cc == 0),
                        stop=(cc == 1),
                    )
            nc.vector.tensor_copy(
                v_sb[:, 2 * p : 2 * p + 2, :DK],
                psv[:, : 2 * DK].rearrange("p (b w) -> p b w", b=2),
            )

        # group g -> prefetch work emitted right after group g's activation
        # (first processed i-block only). qk chunk c covers j-tiles 4c..4c+3
        # (scores of groups 2c, 2c+1); v_pair(p) feeds AV(p) at group p+1.
        pre_slot = {
            0: [lambda: qk_chunk(1), lambda: v_pair(1)],
            1: [lambda: qk_chunk(2), lambda: v_pair(2)],
            2: [lambda: qk_chunk(3), lambda: v_pair(3), lambda: v_pair(4)],
            3: [lambda: qk_chunk(4), lambda: v_pair(5), lambda: v_pair(6)],
            4: [lambda: qk_chunk(5), lambda: v_pair(7)],
            5: [lambda: v_pair(8)],
            6: [lambda: v_pair(9)],
            7: [lambda: v_pair(10)],
        }

        qk_chunk(0)
        v_pair(0)

        # ---- main attention loop ----
        def emit_scores(ps, colb, g, ioff, iw):
            for u in range(2):
                t = 2 * g + u
                lo, hi = (0, 64) if u == 0 else (64, 128)
                kt = qkT2 if u == 0 else qkT
                qt = qkT if u == 0 else qkT2
                nc.tensor.matmul(
                    ps[:, colb + u * 512 : colb + u * 512 + iw],
                    lhsT=kt[lo:hi, t * 128 : (t + 1) * 128],
                    rhs=qt[lo:hi, ioff : ioff + iw],
                    start=True,
                    stop=True,
                    tile_position=(lo, 0),
                )

        def emit_av(pv, ex, base, g, iw, is_first, is_last):
            for u in range(2):
                t = 2 * g + u
                nc.tensor.matmul(
                    pv[:, :iw],
                    lhsT=v_sb[:, t, :],
                    rhs=ex[:, base + u * 512 : base + u * 512 + iw],
                    start=(is_first and u == 0),
                    stop=(is_last and u == 1),
                )

        pending_tail = None
        for ib, (ioff, iw) in enumerate(IBLOCKS):
            pv = pvp.tile([DK + 1, 512], f32, tag="pv", name="pv")
            pending_av = []  # (ex_tile, g)
            for g in range(NG):
                ps = scp.tile([128, 1024], f32, tag="sc", name="sc")
                ex = expp.tile([128, 1024], f16, tag="ex", name="ex")
                emit_scores(ps, 0, g, ioff, iw)
                ps3 = ps.rearrange("p (b w) -> p b w", b=2)[:, :, :iw]
                ex3 = ex.rearrange("p (b w) -> p b w", b=2)[:, :, :iw]
                nc.scalar.activation(
                    out=ex3, in_=ps3, func=Exp, bias=ebias_sb, scale=0.125
                )
                if ib == 0:
                    for work in pre_slot.get(g, ()):
                        work()
                if g == 1 and pending_tail is not None:
                    pending_tail()
                    pending_tail = None
                pending_av.append((ex, g))
                if len(pending_av) > 1:
                    pex, pg = pending_av.pop(0)
                    emit_av(pv, pex, 0, pg, iw, pg == 0, pg == NG - 1)
            for pex, pg in pending_av:
                emit_av(pv, pex, 0, pg, iw, pg == 0, pg == NG - 1)

            res_sb = resp.tile([DK + 1, 512], fr, tag="res", name="res_sb")
            nc.vector.tensor_copy(res_sb[:, :iw], pv[:, :iw])
            nc.gpsimd.dma_start(
                out=lsum[0:1, ioff : ioff + iw],
                in_=res_sb[DK : DK + 1, :iw].bitcast(f32),
            )

            def tail(ioff=ioff, iw=iw, res_sb=res_sb):
                for cc in range(2):
                    po = shp.tile([128, 512], f32, tag="sh", name="po")
                    nc.tensor.matmul(
                        po[:, :iw],
                        lhsT=wo_sb[:, cc * 128 : (cc + 1) * 128],
                        rhs=res_sb[:DK, :iw],
                        start=True,
                        stop=True,
                    )
                    ob = outp.tile([128, 512], f32, tag="ob", name="ob")
                    nc.vector.tensor_copy(ob[:, :iw], po[:, :iw])
                    nc.sync.dma_start(
                        out=out[cc * 128 : (cc + 1) * 128, ioff : ioff + iw],
                        in_=ob[:, :iw],
                    )

            if ib == len(IBLOCKS) - 1:
                tail()
            else:
                pending_tail = tail

    nc.compile()
    return nc


def _get_nc():
    global _NC
    if _NC is None:
        _NC = _build()
    return _NC


def _make_in_maps(inputs):
    x = np.asarray(inputs["x"], dtype=np.float32)
    w_proj = np.asarray(inputs["w_proj"], dtype=np.float32)
    b_proj = np.asarray(inputs["b_proj"], dtype=np.float32)
    w_out = np.asarray(inputs["w_out"], dtype=np.float32)
    in_maps = []
    for core in range(8):
        b, h = divmod(core, H)
        base = h * 3 * DK
        bq = b_proj[base : base + DK]
        bk = b_proj[base + DK : base + 2 * DK]
        in_maps.append(
            {
                "xT": np.ascontiguousarray(x[b].reshape(C, S).astype(np.float16)),
                "wq": np.ascontiguousarray(
                    w_proj[:, base : base + DK].astype(np.float16)
                ),
                "wk": np.ascontiguousarray(
                    w_proj[:, base + DK : base + 2 * DK].astype(np.float16)
                ),
                "wv": np.ascontiguousarray(
                    w_proj[:, base + 2 * DK : base + 3 * DK].astype(np.float16)
                ),
                "bqk": np.ascontiguousarray(
                    np.concatenate([bq, bk]).reshape(128, 1).astype(np.float32)
                ),
                "bkq": np.ascontiguousarray(
                    np.concatenate([bk, bq]).reshape(128, 1).astype(np.float32)
                ),
                "wo": np.ascontiguousarray(w_out[h * DK : (h + 1) * DK, :]),
            }
        )
    return in_maps


def kernel(x, w_proj, b_proj, w_out, b_out):
    from concourse.bass_utils import run_bass_kernel_spmd

    x = np.asarray(x, dtype=np.float32)
    w_proj = np.asarray(w_proj, dtype=np.float32)
    b_proj = np.asarray(b_proj, dtype=np.float32)
    w_out = np.asarray(w_out, dtype=np.float32)
    b_out = np.asarray(b_out, dtype=np.float32)

    B = x.shape[0]
    nc = _get_nc()

    in_maps = _make_in_maps(
        {"x": x, "w_proj": w_proj, "b_proj": b_proj, "w_out": w_out, "b_out": b_out}
    )
    res = run_bass_kernel_spmd(nc, in_maps, list(range(8)))

    outs = np.zeros((B, C, S), dtype=np.float32)
    for b in range(B):
        acc = x[b].reshape(C, S).astype(np.float32) + b_out[:, None]
        for h in range(H):
            core = b * H + h
            dev_o = res.results[core]["out"]  # [C, S] unnormalized
            l = res.results[core]["lsum"]  # [1, S]
            bv = b_proj[h * 3 * DK + 2 * DK : h * 3 * DK + 3 * DK]
            corr = bv @ w_out[h * DK : (h + 1) * DK, :]  # [C]
            acc = acc + dev_o / l + corr[:, None]
        outs[b] = acc
    return outs.reshape(B, C, 14, 14, 14)
